# revision 9
# baseline (speedup 1.0000x reference)
"""Trainium2 Bass kernel for nn_CameraEstimator.

Per batch item b:
    camera[b] = einsum('chw,c->hw', x[b], W)     (C=256 contraction)
    out[b]    = nearest-rotation(camera[b])      (SVD u@vh + det reflection fix)

Contraction (hybrid, per tile of 128x2304):
  - DVE path: fp16 elementwise mult by replicated fp16 W + in-place halving
    fold tree (DVE 2x mode) + grouped fp32 reduce.
  - PE path: fp16 chunk transposes on the PE (identity matmul) -> PSUM,
    copyback (ACT/DVE), 18 accumulating matmuls against a masked fp16 W
    -> camera directly in PSUM.
  x arrives fp16 either via an ACT cast of the fp32 DMA, or (KERNEL_DMACAST=1)
  via a gpsimd SWDGE DMA that casts in flight.

SO(3) projection: closed form via the symmetric eigenproblem of K = Y^T Y.
Eigenvalues from the trig formula (acos via range-reduced atan) + one Newton
polish; then R = Y * f(K) with f(K) = g1*I + d12*(K-l1 I) + d123*(K-l1 I)(K-l2 I),
the quadratic matrix interpolant of g(l_i) = sigma_i/sqrt(l_i) via divided
differences (sigma_3 = sign(det Y) implements the reflection fix).
The K/det front block runs on the Pool engine, the rest on DVE/ACT.

Sharding: batch dim split across 8 NeuronCores (data parallel), W replicated.
"""

import os
import numpy as np

import concourse.bacc as bacc
import concourse.bass as bass
import concourse.mybir as mybir
from concourse.bass_types import AP
from concourse.tile import TileContext
from concourse import bass_utils

F32 = mybir.dt.float32
F16 = mybir.dt.float16
ALU = mybir.AluOpType
ACT = mybir.ActivationFunctionType
AX = mybir.AxisListType

B_FULL = 32768
C = 256
E = 9
CE = C * E                           # 2304
NCH = CE // 128                      # 18 128-wide chunks per tile
N_CORES = 8
P = 128
B_LOCAL = B_FULL // N_CORES          # 4096
TPC = B_LOCAL // P                   # 32 matrices per partition

CHUNKS = [(0, 4), (4, 4), (8, 4), (12, 4), (16, 4), (20, 4), (24, 4),
          (28, 2), (30, 1), (31, 1)]
MAXNT = max(nt for _, nt in CHUNKS)
GROUPS = [(0, 16), (16, 16)]

DMACAST = os.environ.get("KERNEL_DMACAST", "0") == "1"
PE_MOD = int(os.environ.get("KERNEL_PE_MOD", "2"))   # t%PE_MOD!=0 -> DVE path
POLISH2 = os.environ.get("KERNEL_POLISH2", "1") == "1"
POOL_FRONT = os.environ.get("KERNEL_POOL_FRONT", "1") == "1"


def is_pe_tile(t):
    return PE_MOD > 0 and (t % PE_MOD == 1)


def v(base: AP, off: int, *dims) -> AP:
    return AP(base.tensor, base.offset + off,
              [list(base.ap[0])] + [[s, c] for (s, c) in dims])


def make_w16(W: np.ndarray) -> np.ndarray:
    """fp16 W replicated over entries and partitions: [128, 2304]."""
    row = np.repeat(W.astype(np.float16), E)
    return np.ascontiguousarray(np.broadcast_to(row, (P, CE)))


def make_wm9(W: np.ndarray) -> np.ndarray:
    """Masked fp16 W for the PE path: wm9[j, k, e] = fp16(W[(128j+k)//9])
    where (128j+k) % 9 == e, else 0."""
    kidx = np.arange(CE)
    wm = np.zeros((CE, E), np.float32)
    wm[kidx, kidx % E] = W[kidx // E]
    return np.ascontiguousarray(wm.astype(np.float16).reshape(NCH, P, E))


def make_idt() -> np.ndarray:
    return np.ascontiguousarray(np.eye(P, dtype=np.float16))


def _emit(nc, tc, x_ap, w16_ap, wm9_ap, idt_ap, y_ap):
    f32 = F32
    vec = nc.vector
    act = nc.scalar
    pool = nc.gpsimd
    STAGE = int(os.environ.get("KERNEL_STAGE", "99"))

    x_flat = x_ap.rearrange("b c h w -> b (c h w)")
    x_tiled = x_flat.rearrange("(p t) f -> p t f", p=P)
    y_flat = y_ap.rearrange("b h w -> b (h w)").rearrange("(p t) e -> p (t e)", p=P)

    with tc.tile_pool(name="xin", bufs=2) as xpool, \
         tc.tile_pool(name="wk", bufs=1) as wp, \
         tc.tile_pool(name="xtp", bufs=2) as xtpool, \
         tc.tile_pool(name="tp", bufs=2, space="PSUM") as tpp, \
         tc.tile_pool(name="pcp", bufs=4, space="PSUM") as pcp, \
         tc.tile_pool(name="tail", bufs=2) as tp:

        # ---- persistent inputs ------------------------------------------
        w16 = wp.tile([P, CE], F16)
        nc.sync.dma_start(out=w16[:], in_=AP(w16_ap.tensor, 0, [[CE, P], [1, CE]]))
        wm9 = wp.tile([P, NCH * E], F16)
        nc.sync.dma_start(
            out=wm9[:],
            in_=AP(wm9_ap.tensor, 0, [[E, P], [E * P, NCH], [1, E]]))
        idt = wp.tile([P, P], F16)
        nc.sync.dma_start(out=idt[:], in_=AP(idt_ap.tensor, 0, [[P, P], [1, P]]))

        cam = wp.tile([P, TPC * E], f32)

        _consts = {}

        def cb(val):
            if val not in _consts:
                ct = wp.tile([P, 1], f32, name=f"const{len(_consts)}")
                vec.memset(ct[:], float(val))
                _consts[val] = ct[:]
            return _consts[val]

        # ---- contraction -------------------------------------------------
        cb_flip = [0]

        def emit_chunk(t0, nt):
            ncol = nt * CE
            if DMACAST:
                x16 = xpool.tile([P, MAXNT * CE], F16, tag="x16", name=f"x16_{t0}")
                nc.gpsimd.dma_start(out=x16[:, :ncol], in_=x_tiled[:, t0:t0 + nt, :])
            else:
                xt = xpool.tile([P, MAXNT * CE], F32, tag="xt", name=f"xt{t0}")
                nc.sync.dma_start(out=xt[:, :ncol], in_=x_tiled[:, t0:t0 + nt, :])
                x16 = xpool.tile([P, MAXNT * CE], F16, tag="x16", name=f"x16_{t0}")
                act.copy(x16[:, :ncol], xt[:, :ncol])

            dve_tiles = [i for i in range(nt) if not is_pe_tile(t0 + i)]
            pe_tiles = [i for i in range(nt) if is_pe_tile(t0 + i)]

            # DVE path: mult+folds in place on the dve tiles' regions
            for i in dve_tiles:
                off = i * CE
                vec.tensor_tensor(v(x16, off, (1, CE)), v(x16, off, (1, CE)),
                                  v(w16, 0, (1, CE)), ALU.mult)
                n = CE
                while n > 72:
                    n //= 2
                    vec.tensor_tensor(v(x16, off, (1, n)), v(x16, off, (1, n)),
                                      v(x16, off + n, (1, n)), ALU.add)
                t = t0 + i
                vec.tensor_reduce(v(cam, t * E, (1, E)),
                                  v(x16, off, (1, E), (E, 8)),
                                  AX.X, ALU.add)

            # PE path: transpose chunks -> PSUM -> copyback -> matmuls
            for i in pe_tiles:
                t = t0 + i
                off = i * CE
                xT = xtpool.tile([P, CE], F16, tag="xT", name=f"xT{t}")
                for g, (c0, nch) in enumerate(((0, 8), (8, 8), (16, 2))):
                    pt = tpp.tile([P, 1024], F16, tag="pt", name=f"pt{t}_{g}")
                    for a in range(nch):
                        j = c0 + a
                        nc.tensor.transpose(pt[:, P * a:P * (a + 1)],
                                            v(x16, off + P * j, (1, P)), idt[:])
                    if cb_flip[0] % 2 == 0:
                        act.copy(xT[:, P * c0:P * (c0 + nch)], pt[:, :P * nch])
                    else:
                        vec.tensor_copy(xT[:, P * c0:P * (c0 + nch)],
                                        pt[:, :P * nch])
                    cb_flip[0] += 1
                pc = pcp.tile([P, E], f32, tag="pc", name=f"pc{t}")
                for j in range(NCH):
                    nc.tensor.matmul(pc[:], xT[:, P * j:P * (j + 1)],
                                     wm9[:, E * j:E * (j + 1)],
                                     start=(j == 0), stop=(j == NCH - 1))
                act.copy(v(cam, t * E, (1, E)), pc[:, :])

        def emit_group(gi, t0, G):
            NE = G * E

            def mat(tile, off=0):
                return v(tile, off, (E, G), (3, 3), (1, 3))

            def flat(tile):
                return v(tile, 0, (1, NE))

            def diag(tile):
                return v(tile, 0, (E, G), (4, 3))

            def pl(tile):
                return v(tile, 0, (1, G))

            def bc9(tile):
                return v(tile, 0, (1, G), (0, E))

            def bc3(tile):
                return v(tile, 0, (1, G), (0, 3))

            def T(name, cols):
                return tp.tile([P, cols], f32, tag=name, name=f"{name}_{gi}")

            K = T("K", NE)
            Bm = T("Bm", NE)
            Mt = T("Mt", NE)
            t1 = T("t1", NE)
            Rt = T("Rt", NE)
            D2 = T("D2", G * 12)

            def plane(name):
                return T("p_" + name, G)

            c2 = plane("c2"); c2sq = plane("c2sq"); k2 = plane("k2")
            c1 = plane("c1"); c0 = plane("c0"); dety = plane("dety")
            q = plane("q"); p2 = plane("p2"); pp = plane("pp")
            detb = plane("detb"); r = plane("r"); w1 = plane("w1")
            w2 = plane("w2"); w3 = plane("w3"); w4 = plane("w4")
            th = plane("th"); l1 = plane("l1"); l2 = plane("l2"); l3 = plane("l3")
            a1 = plane("a1"); a2 = plane("a2"); a3 = plane("a3")
            al1 = plane("al1"); al2 = plane("al2"); al3 = plane("al3")
            d12 = plane("d12"); d23 = plane("d23"); d123 = plane("d123")
            sneg = plane("sneg"); td = T("p_td", 6 * G)

            zb = v(cb(0.0), 0, (0, G))
            fr = pool if POOL_FRONT else vec      # front-block engine

            # K = Y^T Y
            for k in range(3):
                a = v(cam, t0 * E + 3 * k, (E, G), (1, 3), (0, 3))
                b = v(cam, t0 * E + 3 * k, (E, G), (0, 3), (1, 3))
                if k == 0:
                    fr.tensor_tensor(mat(K), a, b, ALU.mult)
                else:
                    fr.tensor_tensor(mat(t1), a, b, ALU.mult)
                    fr.tensor_tensor(mat(K), mat(K), mat(t1), ALU.add)
            vec.tensor_reduce(pl(c2), diag(K), AX.X, ALU.add)
            fr.tensor_tensor(flat(t1), flat(K), flat(K), ALU.mult)
            vec.tensor_reduce(pl(k2), v(t1, 0, (E, G), (1, E)), AX.X, ALU.add)
            vec.tensor_tensor(pl(c2sq), pl(c2), pl(c2), ALU.mult)
            vec.tensor_tensor(pl(c1), pl(c2sq), pl(k2), ALU.subtract)
            vec.tensor_scalar_mul(pl(c1), pl(c1), 0.5)
            # det Y
            for rep in range(2):
                fr.tensor_copy(v(D2, 3 * rep, (12, G), (6, 2), (1, 3)),
                               v(cam, t0 * E + 3, (E, G), (3, 2), (1, 3)))
            fr.tensor_tensor(v(td, 0, (3, G), (1, 3)),
                             v(D2, 1, (12, G), (1, 3)),
                             v(D2, 8, (12, G), (1, 3)), ALU.mult)
            fr.tensor_tensor(v(td, 3 * G, (3, G), (1, 3)),
                             v(D2, 2, (12, G), (1, 3)),
                             v(D2, 7, (12, G), (1, 3)), ALU.mult)
            fr.tensor_tensor(v(td, 0, (3, G), (1, 3)),
                             v(td, 0, (3, G), (1, 3)),
                             v(td, 3 * G, (3, G), (1, 3)), ALU.subtract)
            fr.tensor_tensor(v(td, 0, (3, G), (1, 3)),
                             v(td, 0, (3, G), (1, 3)),
                             v(cam, t0 * E, (E, G), (1, 3)), ALU.mult)
            vec.tensor_reduce(pl(dety), v(td, 0, (3, G), (1, 3)), AX.X, ALU.add)
            vec.tensor_tensor(pl(c0), pl(dety), pl(dety), ALU.mult)
            vec.tensor_tensor(pl(sneg), pl(dety), zb, ALU.is_lt)

            # eigenvalues via trig formula (DVE planes + ACT)
            vec.tensor_scalar_mul(pl(q), pl(c2), 1.0 / 3.0)
            vec.tensor_scalar_mul(pl(w1), pl(c1), 1.0 / 3.0)
            vec.scalar_tensor_tensor(pl(p2), pl(c2sq), 1.0 / 9.0, pl(w1),
                                     ALU.mult, ALU.subtract)
            vec.tensor_scalar(pl(p2), pl(p2), 1e-30, None, ALU.max)
            act.activation(pl(pp), pl(p2), ACT.Sqrt, bias=cb(0.0))
            vec.scalar_tensor_tensor(pl(w1), pl(c2), 2.0 / 3.0, pl(q),
                                     ALU.mult, ALU.mult)
            vec.tensor_tensor(pl(w1), pl(w1), pl(c1), ALU.subtract)
            vec.tensor_tensor(pl(w1), pl(w1), pl(q), ALU.mult)
            vec.tensor_tensor(pl(detb), pl(w1), pl(c0), ALU.add)
            vec.tensor_tensor(pl(w1), pl(pp), pl(p2), ALU.mult)
            vec.tensor_scalar(pl(w1), pl(w1), 2.0, 1e-30, ALU.mult, ALU.add)
            vec.reciprocal(pl(w1), pl(w1))
            vec.tensor_tensor(pl(r), pl(detb), pl(w1), ALU.mult)
            vec.tensor_scalar(pl(r), pl(r), -1.0, 1.0, ALU.max, ALU.min)
            vec.tensor_tensor(pl(w1), pl(r), pl(r), ALU.mult)
            act.activation(pl(w1), pl(w1), ACT.Sqrt, scale=-1.0,
                           bias=cb(1.0 + 1e-12))                       # u
            vec.tensor_scalar_mul(pl(w2), pl(r), -1.0)
            vec.tensor_tensor(pl(w2), pl(w2), pl(r), ALU.max)          # |r|
            vec.tensor_tensor(pl(w3), pl(w2), pl(w1), ALU.min)
            vec.tensor_tensor(pl(w4), pl(w2), pl(w1), ALU.max)
            vec.reciprocal(pl(w4), pl(w4))
            vec.tensor_tensor(pl(w3), pl(w3), pl(w4), ALU.mult)
            act.activation(pl(w3), pl(w3), ACT.Arctan, bias=cb(0.0))   # phi
            vec.tensor_tensor(pl(w4), pl(w2), pl(w1), ALU.is_gt)       # g
            vec.tensor_tensor(pl(w2), pl(r), zb, ALU.is_lt)            # s
            vec.tensor_tensor(pl(w1), pl(w2), pl(w4), ALU.mult)        # s*g
            vec.tensor_scalar(pl(w1), pl(w1), np.pi, None, ALU.mult)
            vec.tensor_scalar(pl(th), pl(w4), -np.pi / 2.0, np.pi / 2.0,
                              ALU.mult, ALU.add)
            vec.tensor_tensor(pl(w1), pl(w1), pl(th), ALU.add)         # A
            vec.tensor_scalar(pl(w2), pl(w2), -2.0, 1.0, ALU.mult, ALU.add)
            vec.tensor_scalar(pl(w4), pl(w4), 2.0, -1.0, ALU.mult, ALU.add)
            vec.tensor_tensor(pl(w2), pl(w2), pl(w4), ALU.mult)        # B
            vec.tensor_tensor(pl(w3), pl(w3), pl(w2), ALU.mult)
            vec.tensor_tensor(pl(th), pl(w1), pl(w3), ALU.add)         # acos
            vec.tensor_scalar_mul(pl(w4), pl(pp), 2.0)                 # 2p
            act.activation(pl(w1), pl(th), ACT.Sin, scale=-1.0 / 3.0,
                           bias=cb(np.pi / 2.0))
            act.activation(pl(w2), pl(th), ACT.Sin, scale=1.0 / 3.0,
                           bias=cb(-np.pi / 6.0))
            act.activation(pl(w3), pl(th), ACT.Sin, scale=1.0 / 3.0,
                           bias=cb(np.pi / 6.0))
            vec.tensor_tensor(pl(w1), pl(w1), pl(w4), ALU.mult)
            vec.tensor_tensor(pl(l1), pl(w1), pl(q), ALU.add)
            vec.tensor_tensor(pl(w2), pl(w2), pl(w4), ALU.mult)
            vec.tensor_tensor(pl(l2), pl(w2), pl(q), ALU.add)
            vec.tensor_tensor(pl(w3), pl(w3), pl(w4), ALU.mult)
            vec.tensor_tensor(pl(l3), pl(q), pl(w3), ALU.subtract)

            vec.tensor_scalar_mul(pl(c2sq), pl(c2), 2.0)               # 2c2

            def polish(l, guard):
                vec.scalar_tensor_tensor(pl(w1), pl(l), -1.0, pl(c2),
                                         ALU.mult, ALU.add)            # c2-l
                vec.tensor_tensor(pl(w1), pl(w1), pl(l), ALU.mult)
                vec.tensor_tensor(pl(w1), pl(w1), pl(c1), ALU.subtract)
                vec.tensor_tensor(pl(w1), pl(w1), pl(l), ALU.mult)
                vec.tensor_tensor(pl(w1), pl(w1), pl(c0), ALU.add)     # f
                vec.scalar_tensor_tensor(pl(w2), pl(l), -3.0, pl(c2sq),
                                         ALU.mult, ALU.add)
                vec.tensor_tensor(pl(w2), pl(w2), pl(l), ALU.mult)
                vec.tensor_tensor(pl(w2), pl(w2), pl(c1), ALU.subtract)
                vec.tensor_scalar(pl(w2), pl(w2), guard, None, ALU.add)
                vec.reciprocal(pl(w2), pl(w2))
                vec.tensor_tensor(pl(w1), pl(w1), pl(w2), ALU.mult)
                vec.tensor_tensor(pl(l), pl(l), pl(w1), ALU.subtract)
            polish(l3, -1e-20)
            if POLISH2:
                polish(l2, 1e-20)
            vec.tensor_scalar(pl(l1), pl(l1), 1e-25, None, ALU.max)
            vec.tensor_scalar(pl(l2), pl(l2), 1e-25, None, ALU.max)
            vec.tensor_scalar(pl(l3), pl(l3), 1e-25, None, ALU.max)

            act.activation(pl(al1), pl(l1), ACT.Sqrt, bias=cb(0.0))
            act.activation(pl(al2), pl(l2), ACT.Sqrt, bias=cb(0.0))
            act.activation(pl(al3), pl(l3), ACT.Sqrt, bias=cb(0.0))
            vec.reciprocal(pl(a1), pl(al1))
            vec.reciprocal(pl(a2), pl(al2))
            vec.reciprocal(pl(a3), pl(al3))

            # divided differences (sigma on l3 via sneg select)
            vec.tensor_tensor(pl(w1), pl(al1), pl(al2), ALU.add)
            vec.reciprocal(pl(w1), pl(w1))
            vec.tensor_tensor(pl(w2), pl(a1), pl(a2), ALU.mult)
            vec.scalar_tensor_tensor(pl(d12), pl(w2), -1.0, pl(w1),
                                     ALU.mult, ALU.mult)
            vec.tensor_tensor(pl(w3), pl(al2), pl(al3), ALU.add)
            vec.reciprocal(pl(w4), pl(w3))
            vec.tensor_tensor(pl(w2), pl(a2), pl(a3), ALU.mult)
            vec.scalar_tensor_tensor(pl(d23), pl(w2), -1.0, pl(w4),
                                     ALU.mult, ALU.mult)               # d23p
            vec.tensor_tensor(pl(w2), pl(l2), pl(l3), ALU.subtract)
            vec.tensor_scalar(pl(w2), pl(w2), 1e-20, None, ALU.add)
            vec.reciprocal(pl(w2), pl(w2))
            vec.tensor_tensor(pl(w4), pl(a2), pl(a3), ALU.add)
            vec.tensor_tensor(pl(w4), pl(w4), pl(w2), ALU.mult)        # d23m
            vec.tensor_tensor(pl(w2), pl(w4), pl(d23), ALU.subtract)
            vec.tensor_tensor(pl(w2), pl(w2), pl(sneg), ALU.mult)
            vec.tensor_tensor(pl(d23), pl(d23), pl(w2), ALU.add)
            # w1 still = 1/(al1+al2); w3 = al2+al3
            vec.tensor_tensor(pl(w2), pl(al3), pl(al1), ALU.add)
            vec.tensor_tensor(pl(w3), pl(w3), pl(w2), ALU.mult)
            vec.reciprocal(pl(w3), pl(w3))
            vec.tensor_tensor(pl(w2), pl(a1), pl(a2), ALU.mult)
            vec.tensor_tensor(pl(w2), pl(w2), pl(a3), ALU.mult)
            vec.tensor_tensor(pl(w4), pl(al1), pl(al2), ALU.add)
            vec.tensor_tensor(pl(w4), pl(w4), pl(al3), ALU.add)        # S
            vec.tensor_tensor(pl(w4), pl(w4), pl(w1), ALU.mult)
            vec.tensor_tensor(pl(w4), pl(w4), pl(w3), ALU.mult)
            vec.tensor_tensor(pl(d123), pl(w4), pl(w2), ALU.mult)      # d123p
            vec.tensor_tensor(pl(w2), pl(l1), pl(l3), ALU.subtract)
            vec.tensor_scalar(pl(w2), pl(w2), 1e-20, None, ALU.add)
            vec.reciprocal(pl(w2), pl(w2))
            vec.tensor_tensor(pl(w4), pl(d12), pl(d23), ALU.subtract)
            vec.tensor_tensor(pl(w4), pl(w4), pl(w2), ALU.mult)        # d123m
            vec.tensor_tensor(pl(w4), pl(w4), pl(d123), ALU.subtract)
            vec.tensor_tensor(pl(w4), pl(w4), pl(sneg), ALU.mult)
            vec.tensor_tensor(pl(d123), pl(d123), pl(w4), ALU.add)

            # Phi = d123*(K-l1)(K-l2) + d12*(K-l1) + a1*I ; R = Y @ Phi
            vec.tensor_copy(flat(Bm), flat(K))
            vec.tensor_tensor(diag(K), diag(K), bc3(l1), ALU.subtract)
            vec.tensor_tensor(diag(Bm), diag(Bm), bc3(l2), ALU.subtract)
            for k in range(3):
                a = v(K, k, (E, G), (3, 3), (0, 3))
                b = v(Bm, 3 * k, (E, G), (0, 3), (1, 3))
                if k == 0:
                    vec.tensor_tensor(mat(Mt), a, b, ALU.mult)
                else:
                    vec.tensor_tensor(mat(t1), a, b, ALU.mult)
                    vec.tensor_tensor(mat(Mt), mat(Mt), mat(t1), ALU.add)
            vec.tensor_tensor(flat(Mt), flat(Mt), bc9(d123), ALU.mult)
            vec.tensor_tensor(flat(t1), flat(K), bc9(d12), ALU.mult)
            vec.tensor_tensor(flat(Mt), flat(Mt), flat(t1), ALU.add)
            vec.tensor_tensor(diag(Mt), diag(Mt), bc3(a1), ALU.add)
            for k in range(3):
                a = v(cam, t0 * E + k, (E, G), (3, 3), (0, 3))
                b = v(Mt, 3 * k, (E, G), (0, 3), (1, 3))
                if k == 0:
                    vec.tensor_tensor(mat(Rt), a, b, ALU.mult)
                else:
                    vec.tensor_tensor(mat(t1), a, b, ALU.mult)
                    vec.tensor_tensor(mat(Rt), mat(Rt), mat(t1), ALU.add)

            nc.sync.dma_start(out=v(y_flat, t0 * E, (1, NE)), in_=flat(Rt))

        # ---- emission order: chunks 0-3, group 0 tail, rest, group 1 ----
        group_after = {0: 3, 1: len(CHUNKS) - 1}   # group gi after chunk idx
        for ci, (t0, nt) in enumerate(CHUNKS):
            emit_chunk(t0, nt)
            for gi, (g0, G) in enumerate(GROUPS):
                if group_after.get(gi) == ci and STAGE > 2:
                    emit_group(gi, g0, G)
        if STAGE <= 2:
            nc.sync.dma_start(out=y_flat, in_=v(cam, 0, (1, TPC * E)))


def build(b_local=B_LOCAL):
    nc = bacc.Bacc("TRN2", target_bir_lowering=False, debug=False)
    x = nc.dram_tensor("x", [b_local, C, 3, 3], F32, kind="ExternalInput")
    w16 = nc.dram_tensor("w16", [P, CE], F16, kind="ExternalInput")
    wm9 = nc.dram_tensor("wm9", [NCH, P, E], F16, kind="ExternalInput")
    idt = nc.dram_tensor("idt", [P, P], F16, kind="ExternalInput")
    y = nc.dram_tensor("y", [b_local, 3, 3], F32, kind="ExternalOutput")
    with TileContext(nc) as tc:
        _emit(nc, tc, x.ap(), w16.ap(), wm9.ap(), idt.ap(), y.ap())
    nc.compile()
    return nc


_NC_CACHE = {}


def make_in_maps(x: np.ndarray, W: np.ndarray):
    xs = np.ascontiguousarray(x.reshape(N_CORES, B_LOCAL, C, 3, 3))
    W = np.asarray(W, dtype=np.float32)
    w16 = make_w16(W)
    wm9 = make_wm9(W)
    idt = make_idt()
    return [{"x": xs[i], "w16": w16, "wm9": wm9, "idt": idt}
            for i in range(N_CORES)]


def kernel(x: np.ndarray, W: np.ndarray) -> np.ndarray:
    assert x.shape == (B_FULL, C, 3, 3) and W.shape == (C,)
    if "nc" not in _NC_CACHE:
        _NC_CACHE["nc"] = build()
    nc = _NC_CACHE["nc"]
    in_maps = make_in_maps(x, W)
    res = bass_utils.run_bass_kernel_spmd(nc, in_maps, core_ids=list(range(N_CORES)))
    return np.concatenate([r["y"] for r in res.results], axis=0)


if __name__ == "__main__":
    rng = np.random.default_rng(0)
    x = rng.standard_normal((B_FULL, C, 3, 3), dtype=np.float32)
    W = (rng.standard_normal(C, dtype=np.float32) / np.sqrt(C)).astype(np.float32)
    out = kernel(x=x, W=W)
    print(out.shape, out.dtype)
